# revision 4
# baseline (speedup 1.0000x reference)
"""Multi-head attention (COAMultiHeadAttention) on 8 Trainium2 NeuronCores.

Sharding: batch x head-group (core c: batch c//4, heads 4*(c%4)..: a 256-wide
slice of the model dim). Single fully software-pipelined device program:

  - Token-sliced input DMA + projections so the first exp fires ~16us in
    (vs ~69us when phase A fully precedes attention).
  - Attention runs exp-paced on ScalarE (128 calls x [128,1024], the hard
    wall at ~147us/core). QK matmuls contract over 64 rows, so the head-A
    (PE rows 0-63) / head-B (rows 64-127) pairs execute CONCURRENTLY in
    different PE row-groups (~1.9x measured) - PE has slack under the exp
    wall.
  - That slack is filled with deadline-ordered "filler" quanta: the
    remaining K/Q/V projections and the output projection, paced by a
    virtual per-super-step budget. PSUM: st ping-pong 4 banks + ONE att
    accumulator set (2 banks; PV drains heads sequentially with a deep
    pt-tile pool instead of keeping both heads' accumulators live) +
    2 banks for filler accumulation.
  - Softmax denominators ride as a 65th ones-row in V through PV; the
    reciprocal+broadcast runs through a DRAM bounce per (head, q-half).
  - Output projection is d-major (lhsT = wo^T slices stationary), streamed
    per (d8, 512-token) chunk with its own evac+DMA, for qh0 inside the
    exp window and for qh1 in the tail (evac on the then-idle ScalarE).

Host: shards/transposes inputs, sums the 4 partial (D,T) outputs per batch
in fp32, adds bo.
"""

import os
from collections import deque

import ml_dtypes
import numpy as np

import concourse.bass as bass  # noqa: F401
import concourse.mybir as mybir
import concourse.tile as tile
from concourse import bacc, bass_utils

F32 = mybir.dt.float32
BF16 = mybir.dt.bfloat16
AT = mybir.ActivationFunctionType
ALU = mybir.AluOpType

B = 2
T = 2048
D = 1024
N_HEADS = 16
HEAD_DIM = 64
N_CORES = 8
S = 256
NHL = 4
P = 128
DC = D // P        # 8 contraction chunks
TC = T // P        # 16 key chunks
QH = 1024          # q per super-block
SCALE = 1.0 / np.sqrt(HEAD_DIM)

_CACHE = {}
LAST_STATS = {}


def _build_program():
    nc = bacc.Bacc("TRN2", target_bir_lowering=False, debug=False)

    xq_d = nc.dram_tensor("xq", [P, DC, T], BF16, kind="ExternalInput").ap()
    xk_d = nc.dram_tensor("xk", [P, DC, T], BF16, kind="ExternalInput").ap()
    xv_d = nc.dram_tensor("xv", [P, DC, T], BF16, kind="ExternalInput").ap()
    wqt_d = nc.dram_tensor("wqt", [P, DC, S], BF16, kind="ExternalInput").ap()
    wkt_d = nc.dram_tensor("wkt", [P, DC, S], BF16, kind="ExternalInput").ap()
    wvt_d = nc.dram_tensor("wvt", [P, DC, S], BF16, kind="ExternalInput").ap()
    bq_d = nc.dram_tensor("bq", [P, 2], F32, kind="ExternalInput").ap()
    bk_d = nc.dram_tensor("bk", [P, 2], F32, kind="ExternalInput").ap()
    bv_d = nc.dram_tensor("bv", [P, NHL, HEAD_DIM], F32, kind="ExternalInput").ap()
    wot_d = nc.dram_tensor("wot", [P, 2, D], BF16, kind="ExternalInput").ap()
    out_d = nc.dram_tensor("out_part", [DC, P, T], BF16, kind="ExternalOutput").ap()
    sums_d = nc.dram_tensor("sums_scr", [NHL, T], F32).ap()
    rsums_d = nc.dram_tensor("rsums_scr", [NHL, T], F32).ap()

    with tile.TileContext(nc) as tc:
        _body(tc, xq_d, xk_d, xv_d, wqt_d, wkt_d, wvt_d,
              bq_d, bk_d, bv_d, wot_d, out_d, sums_d, rsums_d)
    nc.compile()
    return nc


def _body(tc, xq_d, xk_d, xv_d, wqt_d, wkt_d, wvt_d, bq_d, bk_d, bv_d, wot_d,
          out_d, sums_d, rsums_d):
    nc = tc.nc
    from contextlib import ExitStack
    with ExitStack() as ctx:
        pers = ctx.enter_context(tc.tile_pool(name="pers", bufs=1))
        xw = ctx.enter_context(tc.tile_pool(name="xw", bufs=1))
        pjp = ctx.enter_context(tc.tile_pool(name="pjp", bufs=2, space="PSUM"))
        stp = ctx.enter_context(tc.tile_pool(name="stp", bufs=1, space="PSUM"))
        attp = ctx.enter_context(tc.tile_pool(name="attp", bufs=1, space="PSUM"))
        ptp = ctx.enter_context(tc.tile_pool(name="ptp", bufs=20))
        asbp = ctx.enter_context(tc.tile_pool(name="asbp", bufs=2))
        rcpp = ctx.enter_context(tc.tile_pool(name="rcpp", bufs=2))
        brdp = ctx.enter_context(tc.tile_pool(name="brdp", bufs=2))
        obp = ctx.enter_context(tc.tile_pool(name="obp", bufs=4))

        kt_sb = pers.tile([P, 2, T], BF16, tag="kt")
        qt_sb = pers.tile([P, 2, T], BF16, tag="qt")
        v_sb = pers.tile([P, TC, NHL, 68], BF16, tag="v")
        attn_sb = pers.tile([P, 2, T], BF16, tag="attn")
        wot_sb = pers.tile([P, 2, D], BF16, tag="wot")
        bq_sb = pers.tile([P, 2], F32, tag="bq")
        bk_sb = pers.tile([P, 2], F32, tag="bk")
        bv_sb = pers.tile([P, NHL, HEAD_DIM], F32, tag="bv")
        zero_sb = pers.tile([P, 1], F32, tag="zero")
        scr_sb = pers.tile([P, 1], F32, tag="scr")

        wq_sb = xw.tile([P, DC, S], BF16, tag="wq")
        wk_sb = xw.tile([P, DC, S], BF16, tag="wk")
        wv_sb = xw.tile([P, DC, S], BF16, tag="wv")
        xq_sb = xw.tile([P, DC, T], BF16, tag="xq")
        xk_sb = xw.tile([P, DC, T], BF16, tag="xk")
        xv_sb = xw.tile([P, DC, T], BF16, tag="xv")

        st_sl = [stp.tile([P, QH], F32, tag=f"st{s}", name=f"st{s}")
                 for s in range(2)]

        # ---- exp table preload + constants (runs during DMA lead-in)
        nc.vector.memset(zero_sb[:], 0.0)
        nc.scalar.activation(scr_sb[:], zero_sb[:], AT.Exp,
                             bias=zero_sb[:, 0:1], scale=1.0)
        nc.vector.memset(v_sb[:, :, :, 64:65], 1.0)

        # ---- DMA emissions, priority order (SP issues sequentially)
        nc.sync.dma_start(bq_sb[:], bq_d[:])
        nc.sync.dma_start(bk_sb[:], bk_d[:])
        nc.sync.dma_start(bv_sb[:], bv_d[:])
        nc.sync.dma_start(wk_sb[:], wkt_d[:])
        nc.sync.dma_start(xk_sb[:, :, 0:512], xk_d[:, :, 0:512])
        nc.sync.dma_start(wq_sb[:], wqt_d[:])
        nc.sync.dma_start(xq_sb[:, :, 0:512], xq_d[:, :, 0:512])
        nc.sync.dma_start(wv_sb[:], wvt_d[:])
        nc.sync.dma_start(xv_sb[:, :, 0:512], xv_d[:, :, 0:512])
        nc.sync.dma_start(xq_sb[:, :, 512:1024], xq_d[:, :, 512:1024])
        nc.sync.dma_start(xk_sb[:, :, 512:1024], xk_d[:, :, 512:1024])
        nc.sync.dma_start(xv_sb[:, :, 512:1024], xv_d[:, :, 512:1024])
        nc.sync.dma_start(xq_sb[:, :, 1024:2048], xq_d[:, :, 1024:2048])
        nc.sync.dma_start(xk_sb[:, :, 1024:2048], xk_d[:, :, 1024:2048])
        nc.sync.dma_start(xv_sb[:, :, 1024:2048], xv_d[:, :, 1024:2048])
        nc.sync.dma_start(wot_sb[:], wot_d[:])

        # ---- emission helpers ------------------------------------------
        # kq projection group: one 512-token n-slice of K^T or Q^T for one
        # head-pair m. 8 accumulating matmuls into a pj bank + bias evac.
        def kq_group(x_sb, w_sb, b_sb, dst, m, n):
            qs = slice(n * 512, (n + 1) * 512)
            quanta = []
            pj_box = {}

            def q_mm(d8):
                def run():
                    if d8 == 0:
                        pj_box["t"] = pjp.tile([P, 512], F32, tag="pj",
                                               name="pj")
                    nc.tensor.matmul(
                        pj_box["t"][:], lhsT=w_sb[:, d8, m * P:(m + 1) * P],
                        rhs=x_sb[:, d8, qs], start=(d8 == 0),
                        stop=(d8 == DC - 1))
                    if d8 == DC - 1:
                        nc.vector.tensor_scalar(
                            dst[:, m, qs], pj_box["t"][:],
                            b_sb[:, m:m + 1], None, op0=ALU.add)
                return run
            for d8 in range(DC):
                quanta.append((q_mm(d8), 230))
            return quanta

        # V projection group for one 128-token chunk t16 -> v_sb row.
        def v_group(t16):
            quanta = []
            pj_box = {}

            def q_mm(d8):
                def run():
                    if d8 == 0:
                        pj_box["t"] = pjp.tile([P, 512], F32, tag="pj",
                                               name="pjv")
                    nc.tensor.matmul(
                        pj_box["t"][:, 0:S],
                        lhsT=xv_sb[:, d8, t16 * P:(t16 + 1) * P],
                        rhs=wv_sb[:, d8, :], start=(d8 == 0),
                        stop=(d8 == DC - 1))
                    if d8 == DC - 1:
                        nc.vector.tensor_tensor(
                            v_sb[:, t16, :, 0:64],
                            pj_box["t"][:, 0:S].rearrange("p (h x) -> p h x",
                                                          h=NHL),
                            bv_sb[:], op=ALU.add)
                        v_ready[t16] = step_box[0]
                return run
            for d8 in range(DC):
                quanta.append((q_mm(d8), 170))
            return quanta

        # Output-projection group: one (d8, 512-token) chunk, d-major.
        # 2 accumulating matmuls (s-chunks) + evac + DMA out.
        def c_group(d8, t2, evac_eng):
            ts = slice(t2 * 512, (t2 + 1) * 512)
            quanta = []
            pj_box = {}

            def q_mm(sc):
                def run():
                    if sc == 0:
                        pj_box["t"] = pjp.tile([P, 512], F32, tag="pj",
                                               name="pjc")
                    nc.tensor.matmul(
                        pj_box["t"][:],
                        lhsT=wot_sb[:, sc, d8 * P:(d8 + 1) * P],
                        rhs=attn_sb[:, sc, ts], start=(sc == 0),
                        stop=(sc == 1))
                    if sc == 1:
                        ob = obp.tile([P, 512], BF16, tag="ob", name="ob")
                        if evac_eng == "scalar":
                            nc.scalar.copy(ob[:], pj_box["t"][:])
                        else:
                            nc.vector.tensor_copy(ob[:], pj_box["t"][:])
                        nc.sync.dma_start(out_d[d8, :, ts], ob[:])
                return run
            quanta.append((q_mm(0), 280))
            quanta.append((q_mm(1), 280))
            return quanta

        # ---- filler state ----------------------------------------------
        work_q = deque()   # (deadline_step, quantum, cost)
        v_ready = {}       # t16 -> step emitted
        step_box = [0]
        norms_done = {0: 0, 1: 0}

        def add_group(deadline, quanta):
            # Stagger deadlines inside a group so mandatory forcing never
            # bursts a whole 8-matmul group into one super-step.
            for j, (q, cost) in enumerate(quanta):
                work_q.append((deadline + j // 3, q, cost))

        def pump(budget_ns, force_deadline=None):
            spent = 0
            while work_q:
                dl, q, cost = work_q[0]
                if force_deadline is not None and dl <= force_deadline:
                    pass  # mandatory
                elif spent + cost > budget_ns:
                    break
                work_q.popleft()
                q()
                spent += cost
            return spent

        # ---- PV drain + per-head epilogue ------------------------------
        pt_tiles = {}
        pv_q = deque()     # (h, qh, i); ONE att set -> strict per-head order
        pv_defer = []      # head-B entries, released when head-A completes
        att_box = {}

        def emit_pv_step(h, qh, i):
            if i == 0:
                att_box["t"] = attp.tile([65, QH], F32, tag="att",
                                         name=f"att_{h}_{qh}")
            att = att_box["t"]
            pt = pt_tiles.pop((h, qh, i))
            for n in range(2):
                ns = slice(n * 512, (n + 1) * 512)
                nc.tensor.matmul(
                    att[:, ns], lhsT=v_sb[:, i, h, 0:65],
                    rhs=pt[:, ns], start=(i == 0), stop=(i == TC - 1))
            if i == TC - 1:
                emit_epilogue(h, qh, att)

        def emit_epilogue(h, qh, att):
            p = h // 2
            hb = h % 2
            q0 = qh * QH
            attsb = asbp.tile([65, QH], F32, tag="attsb", name="attsb")
            nc.vector.tensor_copy(attsb[:], att[:])
            nc.sync.dma_start(sums_d[h:h + 1, q0:q0 + QH], attsb[64:65, :])
            sp = rcpp.tile([P, QH // P], F32, tag="sp", name="sp")
            nc.sync.dma_start(
                sp[:], sums_d[h, q0:q0 + QH].rearrange("(p f) -> p f", p=P))
            rp = rcpp.tile([P, QH // P], F32, tag="rp", name="rp")
            nc.vector.reciprocal(rp[:], sp[:])
            nc.sync.dma_start(
                rsums_d[h, q0:q0 + QH].rearrange("(p f) -> p f", p=P), rp[:])
            rc = brdp.tile([64, QH], F32, tag="rc", name="rc")
            nc.sync.dma_start(
                rc[:], rsums_d[h:h + 1, q0:q0 + QH].broadcast_to((64, QH)))
            nc.vector.tensor_tensor(
                attn_sb[hb * 64:hb * 64 + 64, p, q0:q0 + QH],
                attsb[0:64, :], rc[:], op=ALU.mult)
            norms_done[qh] += 1
            if norms_done[qh] == 4:
                tail = (qh == 1)
                base = step_box[0] + 2
                for j, (d8, t2) in enumerate(
                        [(d, t) for d in range(DC) for t in range(2)]):
                    eng = ("scalar" if j % 2 else "vector") if tail \
                        else "vector"
                    add_group(9999 if tail else base + 1 + j,
                              c_group(d8, qh * 2 + t2, eng))

        def drain_pv(max_steps):
            done = 0
            while done < max_steps and pv_q:
                h, qh, i = pv_q[0]
                if v_ready.get(i) is None or v_ready[i] >= step_box[0]:
                    break
                pv_q.popleft()
                emit_pv_step(h, qh, i)
                done += 1
            return done

        # ---- pre-B: minimum projections for super-block 0 --------------
        add_group(-1, kq_group(xk_sb, wk_sb, bk_sb, kt_sb, 0, 0))
        add_group(-1, kq_group(xq_sb, wq_sb, bq_sb, qt_sb, 0, 0))
        add_group(-1, kq_group(xq_sb, wq_sb, bq_sb, qt_sb, 0, 1))
        pump(1 << 30, force_deadline=-1)

        # ---- filler queue with deadlines (in super-steps 0..63) --------
        add_group(0, v_group(0))
        add_group(2, kq_group(xk_sb, wk_sb, bk_sb, kt_sb, 0, 1))
        add_group(3, v_group(1))
        add_group(4, v_group(2))
        add_group(5, v_group(3))
        add_group(6, kq_group(xk_sb, wk_sb, bk_sb, kt_sb, 0, 2))
        add_group(7, v_group(4))
        add_group(8, v_group(5))
        add_group(9, v_group(6))
        add_group(10, kq_group(xk_sb, wk_sb, bk_sb, kt_sb, 0, 3))
        add_group(11, v_group(7))
        add_group(12, v_group(8))
        add_group(13, kq_group(xq_sb, wq_sb, bq_sb, qt_sb, 0, 2))
        add_group(14, kq_group(xq_sb, wq_sb, bq_sb, qt_sb, 0, 3))
        add_group(16, v_group(9))
        add_group(17, v_group(10))
        add_group(18, v_group(11))
        add_group(20, v_group(12))
        add_group(21, v_group(13))
        add_group(22, v_group(14))
        add_group(23, v_group(15))
        add_group(26, kq_group(xk_sb, wk_sb, bk_sb, kt_sb, 1, 0))
        add_group(28, kq_group(xq_sb, wq_sb, bq_sb, qt_sb, 1, 0))
        add_group(29, kq_group(xq_sb, wq_sb, bq_sb, qt_sb, 1, 1))
        add_group(32, kq_group(xk_sb, wk_sb, bk_sb, kt_sb, 1, 1))
        add_group(36, kq_group(xk_sb, wk_sb, bk_sb, kt_sb, 1, 2))
        add_group(40, kq_group(xk_sb, wk_sb, bk_sb, kt_sb, 1, 3))
        add_group(44, kq_group(xq_sb, wq_sb, bq_sb, qt_sb, 1, 2))
        add_group(45, kq_group(xq_sb, wq_sb, bq_sb, qt_sb, 1, 3))

        # ---- super-blocks ----------------------------------------------
        for sb, (p, qh) in enumerate([(0, 0), (0, 1), (1, 0), (1, 1)]):
            hA, hB = 2 * p, 2 * p + 1
            q0 = qh * QH
            for i in range(TC):
                step = sb * TC + i
                step_box[0] = step
                for n in range(2):
                    ns = slice(n * 512, (n + 1) * 512)
                    qs = slice(q0 + n * 512, q0 + (n + 1) * 512)
                    nc.tensor.matmul(
                        st_sl[0][:, ns],
                        lhsT=kt_sb[0:64, p, i * P:(i + 1) * P],
                        rhs=qt_sb[0:64, p, qs], start=True, stop=True)
                    nc.tensor.matmul(
                        st_sl[1][:, ns],
                        lhsT=kt_sb[64:128, p, i * P:(i + 1) * P],
                        rhs=qt_sb[64:128, p, qs], start=True, stop=True)
                for hb, h in ((0, hA), (1, hB)):
                    pt = ptp.tile([P, QH], BF16, tag="pt", name=f"pt{hb}")
                    nc.scalar.activation(pt[:], st_sl[hb][:], AT.Exp,
                                         bias=zero_sb[:, 0:1],
                                         scale=float(SCALE))
                    pt_tiles[(h, qh, i)] = pt
                    if hb == 0:
                        pv_q.append((h, qh, i))
                    else:
                        pv_defer.append((h, qh, i))
                if i == TC - 1:
                    pv_q.extend(pv_defer)
                    pv_defer.clear()
                # mandatory fillers first, then PV / optional fillers
                spent = pump(0, force_deadline=step + 1)
                backlog = len(pv_q)
                pv_budget = 3 if backlog > 8 else 2
                npv = drain_pv(pv_budget)
                rest = 1700 - spent - npv * 460
                if rest > 0:
                    pump(rest)

        # ---- tail ------------------------------------------------------
        while pv_q:
            step_box[0] += 1
            if drain_pv(8) == 0 and work_q:
                pump(1 << 30, force_deadline=9999)
        pump(1 << 30, force_deadline=9999)


def _shard_inputs(query, key, value, wq, bq, wk, bk, wv, bv, wo):
    """Build the 8 per-core input maps (all host-side numpy)."""
    bf16 = ml_dtypes.bfloat16
    in_maps = []

    def fold_dmajor(a_t, inner):
        return np.ascontiguousarray(
            a_t.reshape(DC, P, inner).transpose(1, 0, 2))

    xs = {}
    for b in range(B):
        for name, x in (("xq", query), ("xk", key), ("xv", value)):
            xt = np.ascontiguousarray(x[b].T).astype(bf16)
            xs[(name, b)] = fold_dmajor(xt, T)

    for c in range(N_CORES):
        b, g = divmod(c, NHL)
        gs = g * S
        wq_g = wq[gs:gs + S]
        wk_g = wk[gs:gs + S]
        wv_g = wv[gs:gs + S]
        wo_g = wo[:, gs:gs + S]
        m = {
            "xq": xs[("xq", b)],
            "xk": xs[("xk", b)],
            "xv": xs[("xv", b)],
            "wqt": fold_dmajor(np.ascontiguousarray(wq_g.T).astype(bf16), S),
            "wkt": fold_dmajor(np.ascontiguousarray(wk_g.T).astype(bf16), S),
            "wvt": fold_dmajor(np.ascontiguousarray(wv_g.T).astype(bf16), S),
            "bq": np.ascontiguousarray(
                bq[gs:gs + S].reshape(2, P).T).astype(np.float32),
            "bk": np.ascontiguousarray(
                bk[gs:gs + S].reshape(2, P).T).astype(np.float32),
            "bv": np.ascontiguousarray(np.broadcast_to(
                bv[gs:gs + S].reshape(NHL, HEAD_DIM), (P, NHL, HEAD_DIM))
            ).astype(np.float32),
            "wot": np.ascontiguousarray(
                wo_g.T.reshape(2, P, D).transpose(1, 0, 2)).astype(bf16),
        }
        in_maps.append(m)
    return in_maps


def _reference_numpy(query, key, value, mask, wq, bq, wk, bk, wv, bv, wo, bo):
    """Pure-numpy fallback for non-trivial masks (never hit for spec inputs)."""
    def lin(x, w, b):
        return np.einsum("btd,od->bto", x, w) + b
    Bq, Tq, _ = query.shape
    Q = lin(query, wq, bq).reshape(Bq, Tq, N_HEADS, HEAD_DIM).transpose(0, 2, 1, 3)
    K = lin(key, wk, bk).reshape(Bq, Tq, N_HEADS, HEAD_DIM).transpose(0, 2, 1, 3)
    V = lin(value, wv, bv).reshape(Bq, Tq, N_HEADS, HEAD_DIM).transpose(0, 2, 1, 3)
    scores = np.einsum("bhqd,bhkd->bhqk", Q, K) * SCALE
    scores = np.where(mask[:, None, :, :] == 0, -np.inf, scores)
    scores = scores - scores.max(axis=-1, keepdims=True)
    e = np.exp(scores)
    probs = e / e.sum(axis=-1, keepdims=True)
    att = np.einsum("bhqk,bhkd->bhqd", probs, V)
    att = att.transpose(0, 2, 1, 3).reshape(Bq, Tq, N_HEADS * HEAD_DIM)
    return (np.einsum("btd,od->bto", att, wo) + bo).astype(np.float32)


def _enable_local_tracing():
    """Register the ctypes NTFF-profile hook and keep artifacts local."""
    import sys
    import types
    try:
        import antenv.axon_hooks  # noqa: F401
    except Exception:
        try:
            from trn_agent_boot.trn_boot import _ntff_profile_via_ctypes
            hook = _ntff_profile_via_ctypes("/opt/axon/libaxon_pjrt.so")
            if hook is None:
                return False
            holder = {"hook": hook}
            m2 = types.ModuleType("antenv.axon_hooks")
            m2.get_axon_ntff_profile_hook = lambda: holder["hook"]
            m2.set_axon_ntff_profile_hook = lambda h: holder.update(hook=h)
            if "antenv" not in sys.modules:
                m1 = types.ModuleType("antenv")
                m1.axon_hooks = m2
                sys.modules["antenv"] = m1
            else:
                sys.modules["antenv"].axon_hooks = m2
            sys.modules["antenv.axon_hooks"] = m2
        except Exception:
            return False
    bass_utils.upload_artifacts = lambda tmpdir: tmpdir
    return True


def kernel(query, key, value, mask, wq, bq, wk, bk, wv, bv, wo, bo):
    query = np.asarray(query, np.float32)
    key = np.asarray(key, np.float32)
    value = np.asarray(value, np.float32)
    wq_, bq_ = np.asarray(wq, np.float32), np.asarray(bq, np.float32)
    wk_, bk_ = np.asarray(wk, np.float32), np.asarray(bk, np.float32)
    wv_, bv_ = np.asarray(wv, np.float32), np.asarray(bv, np.float32)
    wo_, bo_ = np.asarray(wo, np.float32), np.asarray(bo, np.float32)
    mask_np = np.asarray(mask)

    if not np.all(mask_np != 0):
        return _reference_numpy(query, key, value, mask_np, wq_, bq_,
                                wk_, bk_, wv_, bv_, wo_, bo_)

    if "prog" not in _CACHE:
        _CACHE["prog"] = _build_program()
    nc = _CACHE["prog"]

    in_maps = _shard_inputs(query, key, value, wq_, bq_, wk_, bk_, wv_, bv_, wo_)

    trace = os.environ.get("KERNEL_TRACE", "0") == "1"
    kw = {}
    if trace:
        trace = _enable_local_tracing()
        if trace:
            tdir = os.environ.get("KERNEL_TRACE_DIR")
            if tdir:
                os.makedirs(tdir, exist_ok=True)
                kw["tmpdir"] = tdir
    try:
        res = bass_utils.run_bass_kernel_spmd(
            nc, in_maps, core_ids=list(range(N_CORES)), trace=trace, **kw)
    except Exception:
        if not trace:
            raise
        import traceback
        traceback.print_exc()
        res = bass_utils.run_bass_kernel_spmd(
            nc, in_maps, core_ids=list(range(N_CORES)), trace=False)

    LAST_STATS.clear()
    LAST_STATS["exec_time_ns"] = res.exec_time_ns
    LAST_STATS["profile_json"] = res.profile_json
    if res.instructions_and_trace is not None:
        LAST_STATS["trace_url"] = res.instructions_and_trace[1]

    out = np.empty((B, T, D), np.float32)
    for b in range(B):
        acc = np.zeros((D, T), np.float32)
        for g in range(NHL):
            acc += res.results[b * NHL + g]["out_part"].reshape(
                D, T).astype(np.float32)
        out[b] = acc.T + bo_
    return out


# revision 5
# speedup vs baseline: 1.0178x; 1.0178x over previous
"""Multi-head attention (COAMultiHeadAttention) on 8 Trainium2 NeuronCores.

Sharding: batch x head-group (core c: batch c//4, heads 4*(c%4)..: a 256-wide
slice of the model dim). Single fully software-pipelined device program:

  - Token-sliced input DMA + projections so the first exp fires ~16us in
    (vs ~69us when phase A fully precedes attention).
  - Attention runs exp-paced on ScalarE (128 calls x [128,1024], the hard
    wall at ~147us/core). QK matmuls contract over 64 rows, so the head-A
    (PE rows 0-63) / head-B (rows 64-127) pairs execute CONCURRENTLY in
    different PE row-groups (~1.9x measured) - PE has slack under the exp
    wall.
  - That slack is filled with deadline-ordered "filler" quanta: the
    remaining K/Q/V projections and the output projection, paced by a
    virtual per-super-step budget. PSUM: st ping-pong 4 banks + ONE att
    accumulator set (2 banks; PV drains heads sequentially with a deep
    pt-tile pool instead of keeping both heads' accumulators live) +
    2 banks for filler accumulation.
  - Softmax denominators ride as a 65th ones-row in V through PV; the
    reciprocal+broadcast runs through a DRAM bounce per (head, q-half).
  - Output projection is d-major (lhsT = wo^T slices stationary), streamed
    per (d8, 512-token) chunk with its own evac+DMA, for qh0 inside the
    exp window and for qh1 in the tail (evac on the then-idle ScalarE).

Host: shards/transposes inputs, sums the 4 partial (D,T) outputs per batch
in fp32, adds bo.
"""

import os
from collections import deque

import ml_dtypes
import numpy as np

import concourse.bass as bass  # noqa: F401
import concourse.mybir as mybir
import concourse.tile as tile
from concourse import bacc, bass_utils

F32 = mybir.dt.float32
BF16 = mybir.dt.bfloat16
AT = mybir.ActivationFunctionType
ALU = mybir.AluOpType

B = 2
T = 2048
D = 1024
N_HEADS = 16
HEAD_DIM = 64
N_CORES = 8
S = 256
NHL = 4
P = 128
DC = D // P        # 8 contraction chunks
TC = T // P        # 16 key chunks
QH = 1024          # q per super-block
SCALE = 1.0 / np.sqrt(HEAD_DIM)

_CACHE = {}
LAST_STATS = {}


def _build_program():
    nc = bacc.Bacc("TRN2", target_bir_lowering=False, debug=False)

    xq_d = nc.dram_tensor("xq", [P, DC, T], BF16, kind="ExternalInput").ap()
    xk_d = nc.dram_tensor("xk", [P, DC, T], BF16, kind="ExternalInput").ap()
    xv_d = nc.dram_tensor("xv", [P, DC, T], BF16, kind="ExternalInput").ap()
    wqt_d = nc.dram_tensor("wqt", [P, DC, S], BF16, kind="ExternalInput").ap()
    wkt_d = nc.dram_tensor("wkt", [P, DC, S], BF16, kind="ExternalInput").ap()
    wvt_d = nc.dram_tensor("wvt", [P, DC, S], BF16, kind="ExternalInput").ap()
    bq_d = nc.dram_tensor("bq", [P, 2], F32, kind="ExternalInput").ap()
    bk_d = nc.dram_tensor("bk", [P, 2], F32, kind="ExternalInput").ap()
    bv_d = nc.dram_tensor("bv", [P, NHL, HEAD_DIM], F32, kind="ExternalInput").ap()
    wot_d = nc.dram_tensor("wot", [P, 2, D], BF16, kind="ExternalInput").ap()
    out_d = nc.dram_tensor("out_part", [DC, P, T], BF16, kind="ExternalOutput").ap()
    sums_d = nc.dram_tensor("sums_scr", [NHL, T], F32).ap()
    rsums_d = nc.dram_tensor("rsums_scr", [NHL, T], F32).ap()

    with tile.TileContext(nc) as tc:
        _body(tc, xq_d, xk_d, xv_d, wqt_d, wkt_d, wvt_d,
              bq_d, bk_d, bv_d, wot_d, out_d, sums_d, rsums_d)
    nc.compile()
    return nc


def _body(tc, xq_d, xk_d, xv_d, wqt_d, wkt_d, wvt_d, bq_d, bk_d, bv_d, wot_d,
          out_d, sums_d, rsums_d):
    nc = tc.nc
    from contextlib import ExitStack
    with ExitStack() as ctx:
        pers = ctx.enter_context(tc.tile_pool(name="pers", bufs=1))
        xw = ctx.enter_context(tc.tile_pool(name="xw", bufs=1))
        stp = ctx.enter_context(tc.tile_pool(name="stp", bufs=1, space="PSUM"))
        attp = ctx.enter_context(tc.tile_pool(name="attp", bufs=1, space="PSUM"))
        ptp = ctx.enter_context(tc.tile_pool(name="ptp", bufs=20))
        asbp = ctx.enter_context(tc.tile_pool(name="asbp", bufs=2))
        rcpp = ctx.enter_context(tc.tile_pool(name="rcpp", bufs=2))
        brdp = ctx.enter_context(tc.tile_pool(name="brdp", bufs=2))
        obp = ctx.enter_context(tc.tile_pool(name="obp", bufs=4))
        # pjp is opened innermost and closed at the last super-block to make
        # room for a second att accumulator (attB): the final pair's heads
        # then drain together instead of serially in the tail.
        pj_cm = tc.tile_pool(name="pjp", bufs=2, space="PSUM")
        pool_box = {"pj": pj_cm.__enter__(), "attB": None}

        kt_sb = pers.tile([P, 2, T], BF16, tag="kt")
        qt_sb = pers.tile([P, 2, T], BF16, tag="qt")
        v_sb = pers.tile([P, TC, NHL, 68], BF16, tag="v")
        attn_sb = pers.tile([P, 2, T], BF16, tag="attn")
        wot_sb = pers.tile([P, 2, D], BF16, tag="wot")
        bq_sb = pers.tile([P, 2], F32, tag="bq")
        bk_sb = pers.tile([P, 2], F32, tag="bk")
        bv_sb = pers.tile([P, NHL, HEAD_DIM], F32, tag="bv")
        zero_sb = pers.tile([P, 1], F32, tag="zero")
        scr_sb = pers.tile([P, 1], F32, tag="scr")

        wq_sb = xw.tile([P, DC, S], BF16, tag="wq")
        wk_sb = xw.tile([P, DC, S], BF16, tag="wk")
        wv_sb = xw.tile([P, DC, S], BF16, tag="wv")
        xq_sb = xw.tile([P, DC, T], BF16, tag="xq")
        xk_sb = xw.tile([P, DC, T], BF16, tag="xk")
        xv_sb = xw.tile([P, DC, T], BF16, tag="xv")

        st_sl = [stp.tile([P, QH], F32, tag=f"st{s}", name=f"st{s}")
                 for s in range(2)]

        # ---- exp table preload + constants (runs during DMA lead-in)
        nc.vector.memset(zero_sb[:], 0.0)
        nc.scalar.activation(scr_sb[:], zero_sb[:], AT.Exp,
                             bias=zero_sb[:, 0:1], scale=1.0)
        nc.vector.memset(v_sb[:, :, :, 64:65], 1.0)

        # ---- DMA emissions, priority order (SP issues sequentially)
        nc.sync.dma_start(bq_sb[:], bq_d[:])
        nc.sync.dma_start(bk_sb[:], bk_d[:])
        nc.sync.dma_start(bv_sb[:], bv_d[:])
        nc.sync.dma_start(wk_sb[:], wkt_d[:])
        nc.sync.dma_start(xk_sb[:, :, 0:512], xk_d[:, :, 0:512])
        nc.sync.dma_start(wq_sb[:], wqt_d[:])
        nc.sync.dma_start(xq_sb[:, :, 0:512], xq_d[:, :, 0:512])
        nc.sync.dma_start(xq_sb[:, :, 512:1024], xq_d[:, :, 512:1024])
        nc.sync.dma_start(wv_sb[:], wvt_d[:])
        nc.sync.dma_start(xv_sb[:, :, 0:512], xv_d[:, :, 0:512])
        nc.sync.dma_start(xk_sb[:, :, 512:1024], xk_d[:, :, 512:1024])
        nc.sync.dma_start(xv_sb[:, :, 512:1024], xv_d[:, :, 512:1024])
        nc.sync.dma_start(xq_sb[:, :, 1024:2048], xq_d[:, :, 1024:2048])
        nc.sync.dma_start(xk_sb[:, :, 1024:2048], xk_d[:, :, 1024:2048])
        nc.sync.dma_start(xv_sb[:, :, 1024:2048], xv_d[:, :, 1024:2048])
        nc.sync.dma_start(wot_sb[:], wot_d[:])

        # Clock-warming junk matmuls: start the PE p-state ramp as soon as
        # wk lands (~5us), so the pre-B projections run near full clock.
        warm = pool_box["pj"].tile([P, 512], F32, tag="pj", name="warm")
        for w in range(10):
            nc.tensor.matmul(warm[:, 0:S], lhsT=wk_sb[:, 0, 0:P],
                             rhs=wk_sb[:, 0, 0:S], start=True, stop=True)

        # ---- emission helpers ------------------------------------------
        # kq projection group: one 512-token n-slice of K^T or Q^T for one
        # head-pair m. 8 accumulating matmuls into a pj bank + bias evac.
        def kq_group(x_sb, w_sb, b_sb, dst, m, n):
            qs = slice(n * 512, (n + 1) * 512)
            quanta = []
            pj_box = {}

            def q_mm(d8):
                def run():
                    if d8 == 0:
                        pj_box["t"] = pool_box["pj"].tile([P, 512], F32, tag="pj",
                                               name="pj")
                    nc.tensor.matmul(
                        pj_box["t"][:], lhsT=w_sb[:, d8, m * P:(m + 1) * P],
                        rhs=x_sb[:, d8, qs], start=(d8 == 0),
                        stop=(d8 == DC - 1))
                    if d8 == DC - 1:
                        nc.vector.tensor_scalar(
                            dst[:, m, qs], pj_box["t"][:],
                            b_sb[:, m:m + 1], None, op0=ALU.add)
                return run
            for d8 in range(DC):
                quanta.append((q_mm(d8), 230))
            return quanta

        # V projection group for one 128-token chunk t16 -> v_sb row.
        def v_group(t16):
            quanta = []
            pj_box = {}

            def q_mm(d8):
                def run():
                    if d8 == 0:
                        pj_box["t"] = pool_box["pj"].tile([P, 512], F32, tag="pj",
                                               name="pjv")
                    nc.tensor.matmul(
                        pj_box["t"][:, 0:S],
                        lhsT=xv_sb[:, d8, t16 * P:(t16 + 1) * P],
                        rhs=wv_sb[:, d8, :], start=(d8 == 0),
                        stop=(d8 == DC - 1))
                    if d8 == DC - 1:
                        nc.vector.tensor_tensor(
                            v_sb[:, t16, :, 0:64],
                            pj_box["t"][:, 0:S].rearrange("p (h x) -> p h x",
                                                          h=NHL),
                            bv_sb[:], op=ALU.add)
                        v_ready[t16] = step_box[0]
                return run
            for d8 in range(DC):
                quanta.append((q_mm(d8), 170))
            return quanta

        # Output-projection group: one (d8, 512-token) chunk, d-major.
        # 2 accumulating matmuls (s-chunks) + evac + DMA out.
        def c_group(d8, t2, evac_eng, gidx=None):
            ts = slice(t2 * 512, (t2 + 1) * 512)
            quanta = []
            pj_box = {}

            def q_mm(sc):
                def run():
                    if sc == 0:
                        if gidx is None:
                            pj_box["t"] = pool_box["pj"].tile(
                                [P, 512], F32, tag="pj", name="pjc")
                        else:
                            # tail: pjp is closed; accumulate in the idle
                            # st slots (4 rotating half-banks)
                            sl = st_sl[gidx % 2]
                            half = (gidx // 2) % 2
                            pj_box["t"] = sl[:, half * 512:half * 512 + 512]
                    nc.tensor.matmul(
                        pj_box["t"][:],
                        lhsT=wot_sb[:, sc, d8 * P:(d8 + 1) * P],
                        rhs=attn_sb[:, sc, ts], start=(sc == 0),
                        stop=(sc == 1))
                    if sc == 1:
                        ob = obp.tile([P, 512], BF16, tag="ob", name="ob")
                        if evac_eng == "scalar":
                            nc.scalar.copy(ob[:], pj_box["t"][:])
                        else:
                            nc.vector.tensor_copy(ob[:], pj_box["t"][:])
                        nc.sync.dma_start(out_d[d8, :, ts], ob[:])
                return run
            quanta.append((q_mm(0), 280))
            quanta.append((q_mm(1), 280))
            return quanta

        # ---- filler state ----------------------------------------------
        work_q = deque()   # (deadline_step, quantum, cost)
        v_ready = {}       # t16 -> step emitted
        step_box = [0]
        norms_done = {0: 0, 1: 0}

        def add_group(deadline, quanta):
            # Stagger deadlines inside a group so mandatory forcing never
            # bursts a whole 8-matmul group into one super-step.
            for j, (q, cost) in enumerate(quanta):
                work_q.append((deadline + j // 3, q, cost))

        def pump(budget_ns, force_deadline=None):
            spent = 0
            while work_q:
                dl, q, cost = work_q[0]
                if force_deadline is not None and dl <= force_deadline:
                    pass  # mandatory
                elif spent + cost > budget_ns:
                    break
                work_q.popleft()
                q()
                spent += cost
            return spent

        # ---- PV drain + per-head epilogue ------------------------------
        pt_tiles = {}
        pv_q = deque()     # (h, qh, i); ONE att set -> strict per-head order
        pv_defer = []      # head-B entries, released when head-A completes
        att_box = {}

        def emit_pv_step(h, qh, i):
            if i == 0:
                pool = pool_box["attB"] if (h, qh) == (3, 1) else attp
                att_box[(h, qh)] = pool.tile([65, QH], F32, tag="att",
                                             name=f"att_{h}_{qh}")
            att = att_box[(h, qh)]
            pt = pt_tiles.pop((h, qh, i))
            for n in range(2):
                ns = slice(n * 512, (n + 1) * 512)
                nc.tensor.matmul(
                    att[:, ns], lhsT=v_sb[:, i, h, 0:65],
                    rhs=pt[:, ns], start=(i == 0), stop=(i == TC - 1))
            if i == TC - 1:
                emit_epilogue(h, qh, att)

        def emit_epilogue(h, qh, att):
            p = h // 2
            hb = h % 2
            q0 = qh * QH
            attsb = asbp.tile([65, QH], F32, tag="attsb", name="attsb")
            nc.vector.tensor_copy(attsb[:], att[:])
            nc.sync.dma_start(sums_d[h:h + 1, q0:q0 + QH], attsb[64:65, :])
            sp = rcpp.tile([P, QH // P], F32, tag="sp", name="sp")
            nc.sync.dma_start(
                sp[:], sums_d[h, q0:q0 + QH].rearrange("(p f) -> p f", p=P))
            rp = rcpp.tile([P, QH // P], F32, tag="rp", name="rp")
            nc.vector.reciprocal(rp[:], sp[:])
            nc.sync.dma_start(
                rsums_d[h, q0:q0 + QH].rearrange("(p f) -> p f", p=P), rp[:])
            rc = brdp.tile([64, QH], F32, tag="rc", name="rc")
            nc.sync.dma_start(
                rc[:], rsums_d[h:h + 1, q0:q0 + QH].broadcast_to((64, QH)))
            nc.vector.tensor_tensor(
                attn_sb[hb * 64:hb * 64 + 64, p, q0:q0 + QH],
                attsb[0:64, :], rc[:], op=ALU.mult)
            norms_done[qh] += 1
            if norms_done[qh] == 4:
                tail = (qh == 1)
                base = step_box[0] + 2
                for j, (d8, t2) in enumerate(
                        [(d, t) for d in range(DC) for t in range(2)]):
                    eng = ("scalar" if j % 2 else "vector") if tail \
                        else "vector"
                    add_group(9999 if tail else base + 1 + j,
                              c_group(d8, qh * 2 + t2, eng,
                                      gidx=j if tail else None))

        def drain_pv(max_steps):
            done = 0
            while done < max_steps and pv_q:
                h, qh, i = pv_q[0]
                if v_ready.get(i) is None or v_ready[i] >= step_box[0]:
                    break
                pv_q.popleft()
                emit_pv_step(h, qh, i)
                done += 1
            return done

        # ---- pre-B: minimum projections for super-block 0 --------------
        add_group(-1, kq_group(xk_sb, wk_sb, bk_sb, kt_sb, 0, 0))
        add_group(-1, kq_group(xq_sb, wq_sb, bq_sb, qt_sb, 0, 0))
        add_group(-1, kq_group(xq_sb, wq_sb, bq_sb, qt_sb, 0, 1))
        pump(1 << 30, force_deadline=-1)

        # ---- filler queue with deadlines (in super-steps 0..63) --------
        add_group(0, v_group(0))
        add_group(2, kq_group(xk_sb, wk_sb, bk_sb, kt_sb, 0, 1))
        add_group(3, v_group(1))
        add_group(4, v_group(2))
        add_group(5, v_group(3))
        add_group(6, kq_group(xk_sb, wk_sb, bk_sb, kt_sb, 0, 2))
        add_group(7, v_group(4))
        add_group(8, v_group(5))
        add_group(9, v_group(6))
        add_group(10, kq_group(xk_sb, wk_sb, bk_sb, kt_sb, 0, 3))
        add_group(11, v_group(7))
        add_group(12, kq_group(xq_sb, wq_sb, bq_sb, qt_sb, 1, 0))
        add_group(13, kq_group(xq_sb, wq_sb, bq_sb, qt_sb, 1, 1))
        add_group(14, kq_group(xk_sb, wk_sb, bk_sb, kt_sb, 1, 0))
        add_group(16, v_group(8))
        add_group(17, v_group(9))
        add_group(18, kq_group(xk_sb, wk_sb, bk_sb, kt_sb, 1, 1))
        add_group(19, v_group(10))
        add_group(20, v_group(11))
        add_group(21, kq_group(xk_sb, wk_sb, bk_sb, kt_sb, 1, 2))
        add_group(22, v_group(12))
        add_group(23, v_group(13))
        add_group(24, kq_group(xk_sb, wk_sb, bk_sb, kt_sb, 1, 3))
        add_group(25, v_group(14))
        add_group(26, v_group(15))
        add_group(29, kq_group(xq_sb, wq_sb, bq_sb, qt_sb, 0, 2))
        add_group(30, kq_group(xq_sb, wq_sb, bq_sb, qt_sb, 0, 3))
        add_group(44, kq_group(xq_sb, wq_sb, bq_sb, qt_sb, 1, 2))
        add_group(45, kq_group(xq_sb, wq_sb, bq_sb, qt_sb, 1, 3))

        # ---- super-blocks ----------------------------------------------
        for sb, (p, qh) in enumerate([(0, 0), (1, 0), (0, 1), (1, 1)]):
            hA, hB = 2 * p, 2 * p + 1
            q0 = qh * QH
            if sb == 3:
                # everything that accumulates in pjp must be done by now
                pump(1 << 30, force_deadline=9999)
                pj_cm.__exit__(None, None, None)
                pool_box["attB"] = ctx.enter_context(
                    tc.tile_pool(name="attB", bufs=1, space="PSUM"))
            for i in range(TC):
                step = sb * TC + i
                step_box[0] = step
                for n in range(2):
                    ns = slice(n * 512, (n + 1) * 512)
                    qs = slice(q0 + n * 512, q0 + (n + 1) * 512)
                    nc.tensor.matmul(
                        st_sl[0][:, ns],
                        lhsT=kt_sb[0:64, p, i * P:(i + 1) * P],
                        rhs=qt_sb[0:64, p, qs], start=True, stop=True)
                    nc.tensor.matmul(
                        st_sl[1][:, ns],
                        lhsT=kt_sb[64:128, p, i * P:(i + 1) * P],
                        rhs=qt_sb[64:128, p, qs], start=True, stop=True)
                for hb, h in ((0, hA), (1, hB)):
                    pt = ptp.tile([P, QH], BF16, tag="pt", name=f"pt{hb}")
                    nc.scalar.activation(pt[:], st_sl[hb][:], AT.Exp,
                                         bias=zero_sb[:, 0:1],
                                         scale=float(SCALE))
                    pt_tiles[(h, qh, i)] = pt
                    if hb == 0 or sb == 3:
                        # sb3: head B drains interleaved into its own
                        # accumulator (attB) - no defer needed
                        pv_q.append((h, qh, i))
                    else:
                        pv_defer.append((h, qh, i))
                if i == TC - 1:
                    pv_q.extend(pv_defer)
                    pv_defer.clear()
                # mandatory fillers first, then PV / optional fillers
                spent = pump(0, force_deadline=step + 1)
                backlog = len(pv_q)
                pv_budget = 4 if backlog > 12 else (3 if backlog > 6 else 2)
                npv = drain_pv(pv_budget)
                rest = 1700 - spent - npv * 460
                if rest > 0:
                    pump(rest)

        # ---- tail ------------------------------------------------------
        while pv_q:
            step_box[0] += 1
            if drain_pv(8) == 0 and work_q:
                pump(1 << 30, force_deadline=9999)
        pump(1 << 30, force_deadline=9999)


def _shard_inputs(query, key, value, wq, bq, wk, bk, wv, bv, wo):
    """Build the 8 per-core input maps (all host-side numpy)."""
    bf16 = ml_dtypes.bfloat16
    in_maps = []

    def fold_dmajor(a_t, inner):
        return np.ascontiguousarray(
            a_t.reshape(DC, P, inner).transpose(1, 0, 2))

    xs = {}
    for b in range(B):
        for name, x in (("xq", query), ("xk", key), ("xv", value)):
            xt = np.ascontiguousarray(x[b].T).astype(bf16)
            xs[(name, b)] = fold_dmajor(xt, T)

    for c in range(N_CORES):
        b, g = divmod(c, NHL)
        gs = g * S
        wq_g = wq[gs:gs + S]
        wk_g = wk[gs:gs + S]
        wv_g = wv[gs:gs + S]
        wo_g = wo[:, gs:gs + S]
        m = {
            "xq": xs[("xq", b)],
            "xk": xs[("xk", b)],
            "xv": xs[("xv", b)],
            "wqt": fold_dmajor(np.ascontiguousarray(wq_g.T).astype(bf16), S),
            "wkt": fold_dmajor(np.ascontiguousarray(wk_g.T).astype(bf16), S),
            "wvt": fold_dmajor(np.ascontiguousarray(wv_g.T).astype(bf16), S),
            "bq": np.ascontiguousarray(
                bq[gs:gs + S].reshape(2, P).T).astype(np.float32),
            "bk": np.ascontiguousarray(
                bk[gs:gs + S].reshape(2, P).T).astype(np.float32),
            "bv": np.ascontiguousarray(np.broadcast_to(
                bv[gs:gs + S].reshape(NHL, HEAD_DIM), (P, NHL, HEAD_DIM))
            ).astype(np.float32),
            "wot": np.ascontiguousarray(
                wo_g.T.reshape(2, P, D).transpose(1, 0, 2)).astype(bf16),
        }
        in_maps.append(m)
    return in_maps


def _reference_numpy(query, key, value, mask, wq, bq, wk, bk, wv, bv, wo, bo):
    """Pure-numpy fallback for non-trivial masks (never hit for spec inputs)."""
    def lin(x, w, b):
        return np.einsum("btd,od->bto", x, w) + b
    Bq, Tq, _ = query.shape
    Q = lin(query, wq, bq).reshape(Bq, Tq, N_HEADS, HEAD_DIM).transpose(0, 2, 1, 3)
    K = lin(key, wk, bk).reshape(Bq, Tq, N_HEADS, HEAD_DIM).transpose(0, 2, 1, 3)
    V = lin(value, wv, bv).reshape(Bq, Tq, N_HEADS, HEAD_DIM).transpose(0, 2, 1, 3)
    scores = np.einsum("bhqd,bhkd->bhqk", Q, K) * SCALE
    scores = np.where(mask[:, None, :, :] == 0, -np.inf, scores)
    scores = scores - scores.max(axis=-1, keepdims=True)
    e = np.exp(scores)
    probs = e / e.sum(axis=-1, keepdims=True)
    att = np.einsum("bhqk,bhkd->bhqd", probs, V)
    att = att.transpose(0, 2, 1, 3).reshape(Bq, Tq, N_HEADS * HEAD_DIM)
    return (np.einsum("btd,od->bto", att, wo) + bo).astype(np.float32)


def _enable_local_tracing():
    """Register the ctypes NTFF-profile hook and keep artifacts local."""
    import sys
    import types
    try:
        import antenv.axon_hooks  # noqa: F401
    except Exception:
        try:
            from trn_agent_boot.trn_boot import _ntff_profile_via_ctypes
            hook = _ntff_profile_via_ctypes("/opt/axon/libaxon_pjrt.so")
            if hook is None:
                return False
            holder = {"hook": hook}
            m2 = types.ModuleType("antenv.axon_hooks")
            m2.get_axon_ntff_profile_hook = lambda: holder["hook"]
            m2.set_axon_ntff_profile_hook = lambda h: holder.update(hook=h)
            if "antenv" not in sys.modules:
                m1 = types.ModuleType("antenv")
                m1.axon_hooks = m2
                sys.modules["antenv"] = m1
            else:
                sys.modules["antenv"].axon_hooks = m2
            sys.modules["antenv.axon_hooks"] = m2
        except Exception:
            return False
    bass_utils.upload_artifacts = lambda tmpdir: tmpdir
    return True


def kernel(query, key, value, mask, wq, bq, wk, bk, wv, bv, wo, bo):
    query = np.asarray(query, np.float32)
    key = np.asarray(key, np.float32)
    value = np.asarray(value, np.float32)
    wq_, bq_ = np.asarray(wq, np.float32), np.asarray(bq, np.float32)
    wk_, bk_ = np.asarray(wk, np.float32), np.asarray(bk, np.float32)
    wv_, bv_ = np.asarray(wv, np.float32), np.asarray(bv, np.float32)
    wo_, bo_ = np.asarray(wo, np.float32), np.asarray(bo, np.float32)
    mask_np = np.asarray(mask)

    if not np.all(mask_np != 0):
        return _reference_numpy(query, key, value, mask_np, wq_, bq_,
                                wk_, bk_, wv_, bv_, wo_, bo_)

    if "prog" not in _CACHE:
        _CACHE["prog"] = _build_program()
    nc = _CACHE["prog"]

    in_maps = _shard_inputs(query, key, value, wq_, bq_, wk_, bk_, wv_, bv_, wo_)

    trace = os.environ.get("KERNEL_TRACE", "0") == "1"
    kw = {}
    if trace:
        trace = _enable_local_tracing()
        if trace:
            tdir = os.environ.get("KERNEL_TRACE_DIR")
            if tdir:
                os.makedirs(tdir, exist_ok=True)
                kw["tmpdir"] = tdir
    try:
        res = bass_utils.run_bass_kernel_spmd(
            nc, in_maps, core_ids=list(range(N_CORES)), trace=trace, **kw)
    except Exception:
        if not trace:
            raise
        import traceback
        traceback.print_exc()
        res = bass_utils.run_bass_kernel_spmd(
            nc, in_maps, core_ids=list(range(N_CORES)), trace=False)

    LAST_STATS.clear()
    LAST_STATS["exec_time_ns"] = res.exec_time_ns
    LAST_STATS["profile_json"] = res.profile_json
    if res.instructions_and_trace is not None:
        LAST_STATS["trace_url"] = res.instructions_and_trace[1]

    out = np.empty((B, T, D), np.float32)
    for b in range(B):
        acc = np.zeros((D, T), np.float32)
        for g in range(NHL):
            acc += res.results[b * NHL + g]["out_part"].reshape(
                D, T).astype(np.float32)
        out[b] = acc.T + bo_
    return out
